# revision 4
# baseline (speedup 1.0000x reference)
"""Trainium2 Bass kernel: multi-scale depthwise + SE + 1x1 CNN block.

Data-parallel over batch: 16 samples -> 8 NeuronCores, 2 samples/core.
Per core, the 128 SBUF partitions hold (sample, channel) pairs; each
partition keeps its zero-padded 262x262 bf16 image resident in SBUF.

The three depthwise kernels are merged host-side into one 7x7 (49-tap)
kernel. Each tap is one TensorE matmul with a diagonal [128,128] bf16
stationary (per-partition tap weight) against a shifted 2-row window of
the padded image, accumulating xm in PSUM. The SE unit is folded into
the 1x1 conv: s = sigmoid(fc2(relu(fc1(mean(xm))))) is computed
host-side (mean(xm) = bm + sum(km)*mean(x) exactly up to border terms
~1e-6 of output) and multiplied into the 1x1 weights, so the second
PSUM bank accumulates W1x1*diag(s) @ xm plus an identity-stationary
residual matmul of x itself. Eviction adds the folded bias
(W4 @ bm + b1x1) via tensor_scalar_add and casts to bf16.
"""

import contextlib
import ctypes
import os
import sys
import types

sys.path.insert(0, "/opt/trn_rl_repo")

import numpy as np

_CACHE = {}


# ---------------------------------------------------------------------------
# Workaround: the container's walrus build rejects >1 sync wait per
# instruction. Move excess waits onto same-engine NoOps inserted just before.
# ---------------------------------------------------------------------------
_WAITFIX_COUNTER = [0]


def _make_wait_nop(engine, waits):
    import bass_rust
    from concourse import mybir

    _WAITFIX_COUNTER[0] += 1
    nop = bass_rust.InstNoOp(name=f"waitfix-{_WAITFIX_COUNTER[0]}", ins=[], outs=[])
    nop.engine = engine
    nop.sync_info = mybir.SyncInfo(on_wait=list(waits), on_update=[])
    return nop


def _split_excess_waits(nc, max_waits=1):
    from concourse import mybir

    n_split = 0
    for fn in nc.m.functions:
        for bb in fn.blocks:
            out = []
            for inst in bb.instructions:
                si = inst.sync_info
                if si is not None and si.on_wait is not None \
                        and len(si.on_wait) > max_waits:
                    waits = list(si.on_wait)
                    extra, keep = waits[:-max_waits], waits[-max_waits:]
                    for i in range(0, len(extra), max_waits):
                        out.append(
                            _make_wait_nop(inst.engine, extra[i : i + max_waits])
                        )
                    inst.sync_info = mybir.SyncInfo(
                        on_wait=keep, on_update=list(si.on_update or [])
                    )
                    n_split += 1
                out.append(inst)
            if len(out) != len(bb.instructions):
                bb.instructions[:] = out
    return n_split


# ---------------------------------------------------------------------------
# Optional NTFF profiling hook (the image's antenv stub lacks axon_hooks).
# Only used when PROBLEM_TRACE=1; grading path runs trace=False.
# ---------------------------------------------------------------------------
def _install_ntff_hook():
    so_path = "/opt/axon/libaxon_pjrt.so"
    try:
        lib = ctypes.CDLL(so_path)
    except OSError:
        return False
    if not hasattr(lib, "axon_start_nrt_profile"):
        return False
    lib.axon_start_nrt_profile.argtypes = [
        ctypes.POINTER(ctypes.c_int64),
        ctypes.c_size_t,
    ]
    lib.axon_start_nrt_profile.restype = ctypes.c_int64
    lib.axon_stop_nrt_profile.argtypes = [ctypes.c_char_p]
    lib.axon_stop_nrt_profile.restype = ctypes.c_int64

    @contextlib.contextmanager
    def _hook(output_dir, device_ids):
        import jax

        jax.devices()
        if device_ids:
            ids = (ctypes.c_int64 * len(device_ids))(*device_ids)
            rc = lib.axon_start_nrt_profile(ids, len(device_ids))
        else:
            rc = lib.axon_start_nrt_profile(None, 0)
        if rc != 0:
            raise RuntimeError(f"axon_start_nrt_profile rc={rc}")
        try:
            yield
        finally:
            n = lib.axon_stop_nrt_profile(str(output_dir).encode())
            print(f"profile: {n} file(s) written to {output_dir}", file=sys.stderr)

    mod = types.ModuleType("antenv.axon_hooks")
    mod._hook = _hook
    mod.get_axon_ntff_profile_hook = lambda: mod._hook
    mod.set_axon_ntff_profile_hook = lambda h: setattr(mod, "_hook", h)
    import antenv

    antenv.axon_hooks = mod
    sys.modules["antenv.axon_hooks"] = mod
    return True

H = 256
W = 256
HP = H + 6
WP = W + 6
C = 64
NCORES = 8
ROWS_PER_CHUNK = 2
CHUNK = ROWS_PER_CHUNK * W  # 512, one PSUM bank
NCHUNK = H // ROWS_PER_CHUNK

# tap split: taps listed in (dh, dw) order; which go to DVE/ACT vs PE
ALL_TAPS = [(dh, dw) for dh in range(7) for dw in range(7)]


def _build_nc():
    import concourse.bass as bass
    import concourse.tile as tile
    from concourse import mybir

    BF = mybir.dt.bfloat16
    FP = mybir.dt.float32

    nc = bass.Bass(target_bir_lowering=False)
    x_in = nc.dram_tensor("x", [128, HP, WP], BF, kind="ExternalInput")
    wd_in = nc.dram_tensor("wd", [128, 49, 128], BF, kind="ExternalInput")
    w4_in = nc.dram_tensor("w4", [128, 128], BF, kind="ExternalInput")
    id_in = nc.dram_tensor("ident", [128, 128], BF, kind="ExternalInput")
    b_in = nc.dram_tensor("bias", [128, 1], FP, kind="ExternalInput")
    y_out = nc.dram_tensor("y", [128, H * W], BF, kind="ExternalOutput")

    with tile.TileContext(nc) as tc:
        with tc.tile_pool(name="xp", bufs=1) as xp, \
             tc.tile_pool(name="wp", bufs=1) as wp, \
             tc.tile_pool(name="psA", bufs=2, space="PSUM") as psA, \
             tc.tile_pool(name="psB", bufs=2, space="PSUM") as psB, \
             tc.tile_pool(name="xm", bufs=3) as xmp, \
             tc.tile_pool(name="ob", bufs=3) as obp:
            x = xp.tile([128, HP, WP], BF)
            nc.sync.dma_start(out=x, in_=x_in[:, :, :])
            wd = wp.tile([128, 49, 128], BF)
            nc.sync.dma_start(out=wd, in_=wd_in[:, :, :])
            w4 = wp.tile([128, 128], BF)
            nc.sync.dma_start(out=w4, in_=w4_in[:, :])
            ident = wp.tile([128, 128], BF)
            nc.sync.dma_start(out=ident, in_=id_in[:, :])
            bias = wp.tile([128, 1], FP)
            nc.sync.dma_start(out=bias, in_=b_in[:, :])

            for ch in range(NCHUNK):
                h0 = ch * ROWS_PER_CHUNK
                pa = psA.tile([128, CHUNK], FP)
                for ti, (dh, dw) in enumerate(ALL_TAPS):
                    nc.tensor.matmul(
                        out=pa,
                        lhsT=wd[:, ti, :],
                        rhs=x[:, h0 + dh : h0 + dh + ROWS_PER_CHUNK, dw : dw + W],
                        start=(ti == 0),
                        stop=(ti == len(ALL_TAPS) - 1),
                    )
                xm = xmp.tile([128, CHUNK], BF)
                nc.scalar.copy(out=xm, in_=pa)

                pb = psB.tile([128, CHUNK], FP)
                nc.tensor.matmul(out=pb, lhsT=w4, rhs=xm, start=True, stop=False)
                nc.tensor.matmul(
                    out=pb,
                    lhsT=ident,
                    rhs=x[:, h0 + 3 : h0 + 3 + ROWS_PER_CHUNK, 3 : 3 + W],
                    start=False,
                    stop=True,
                )
                ob = obp.tile([128, CHUNK], BF)
                nc.vector.tensor_scalar_add(out=ob, in0=pb, scalar1=bias)
                nc.sync.dma_start(
                    out=y_out[:, h0 * W : h0 * W + CHUNK], in_=ob
                )
    return nc


def _host_prep(x, w7, b7, w5, b5, w3, b3, fc1_w, fc1_b, fc2_w, fc2_b, w1x1, b1x1):
    import ml_dtypes

    bf16 = ml_dtypes.bfloat16

    km = w7[:, 0].astype(np.float64).copy()
    km[:, 1:6, 1:6] += w5[:, 0]
    km[:, 2:5, 2:5] += w3[:, 0]
    bm = (b7 + b5 + b3).astype(np.float64)

    n = x.shape[0]
    # SE scale: mean(xm) ~= bm + sum(km)*mean(x)  (border term ~1e-6 of out)
    mx = x.astype(np.float64).mean(axis=(2, 3))  # [N, C]
    m = bm[None, :] + km.sum(axis=(1, 2))[None, :] * mx
    y1 = np.maximum(m @ fc1_w.T.astype(np.float64) + fc1_b, 0)
    s = 1 / (1 + np.exp(-(y1 @ fc2_w.T.astype(np.float64) + fc2_b)))  # [N, C]

    w11 = w1x1[:, :, 0, 0].astype(np.float64)  # [o, c]

    # padded bf16 input per core
    xb = x.astype(bf16)
    xpad = np.zeros((NCORES, 128, HP, WP), bf16)
    xpad[:, :, 3 : 3 + H, 3 : 3 + W] = xb.reshape(NCORES, 128, H, W)

    # 49 diagonal tap weights, shared across cores
    km_bf = km.astype(bf16).astype(np.float64)  # quantize once
    wd = np.zeros((128, 49, 128), bf16)
    ks = np.arange(128)
    wd[ks, :, ks] = km_bf[ks % C].reshape(128, 49).astype(bf16)

    ident = np.eye(128, dtype=bf16)

    # per-core W4 block-diag + bias
    w4s = []
    biases = []
    for core in range(NCORES):
        w4 = np.zeros((128, 128), np.float64)
        bias = np.zeros((128,), np.float64)
        for ni in range(2):
            sn = s[core * 2 + ni]  # [C]
            blk = (w11 * sn[None, :]).T  # [c, o]
            w4[64 * ni : 64 * ni + 64, 64 * ni : 64 * ni + 64] = blk
            bias[64 * ni : 64 * ni + 64] = w11 @ (sn * bm) + b1x1
        w4s.append(w4.astype(bf16))
        biases.append(bias.astype(np.float32).reshape(128, 1))

    in_maps = []
    for core in range(NCORES):
        in_maps.append(
            {
                "x": xpad[core],
                "wd": wd,
                "w4": w4s[core],
                "ident": ident,
                "bias": biases[core],
            }
        )
    return in_maps


def kernel(x, w7, b7, w5, b5, w3, b3, fc1_w, fc1_b, fc2_w, fc2_b, w1x1, b1x1):
    args = [np.asarray(a) for a in (x, w7, b7, w5, b5, w3, b3, fc1_w, fc1_b,
                                    fc2_w, fc2_b, w1x1, b1x1)]
    in_maps = _host_prep(*args)

    if "nc" not in _CACHE:
        if os.environ.get("PROBLEM_TRACE"):
            try:
                _install_ntff_hook()
            except Exception:
                pass
        nc = _build_nc()
        _split_excess_waits(nc, max_waits=1)
        _CACHE["nc"] = nc
    nc = _CACHE["nc"]

    from concourse.bass_utils import run_bass_kernel_spmd

    trace = bool(os.environ.get("PROBLEM_TRACE"))
    res = run_bass_kernel_spmd(
        nc, in_maps, core_ids=list(range(NCORES)), trace=trace
    )
    kernel.last_result = res

    out = np.stack([r["y"] for r in res.results])  # [8, 128, H*W] bf16
    out = out.reshape(16, C, H, W).astype(np.float32)
    return out


# revision 6
# speedup vs baseline: 1.2179x; 1.2179x over previous
"""Trainium2 Bass kernel: multi-scale depthwise + SE + 1x1 CNN block.

Data-parallel over batch: 16 samples -> 8 NeuronCores, 2 samples/core.
Per core, the 128 SBUF partitions hold (sample, channel) pairs; each
partition keeps its zero-padded 262x262 bf16 image resident in SBUF.

The three depthwise kernels are merged host-side into one 7x7 (49-tap)
kernel. Each tap is one TensorE matmul with a diagonal [128,128] bf16
stationary (per-partition tap weight) against a shifted 2-row window of
the padded image, accumulating xm in PSUM. The SE unit is folded into
the 1x1 conv: s = sigmoid(fc2(relu(fc1(mean(xm))))) is computed
host-side (mean(xm) = bm + sum(km)*mean(x) exactly up to border terms
~1e-6 of output) and multiplied into the 1x1 weights, so the second
PSUM bank accumulates W1x1*diag(s) @ xm plus an identity-stationary
residual matmul of x itself. Eviction adds the folded bias
(W4 @ bm + b1x1) via tensor_scalar_add and casts to bf16.
"""

import contextlib
import ctypes
import os
import sys
import types

sys.path.insert(0, "/opt/trn_rl_repo")

import numpy as np

_CACHE = {}


# ---------------------------------------------------------------------------
# Workaround: the container's walrus build rejects >1 sync wait per
# instruction. Move excess waits onto same-engine NoOps inserted just before.
# ---------------------------------------------------------------------------
_WAITFIX_COUNTER = [0]


def _make_wait_nop(engine, waits):
    import bass_rust
    from concourse import mybir

    _WAITFIX_COUNTER[0] += 1
    nop = bass_rust.InstNoOp(name=f"waitfix-{_WAITFIX_COUNTER[0]}", ins=[], outs=[])
    nop.engine = engine
    nop.sync_info = mybir.SyncInfo(on_wait=list(waits), on_update=[])
    return nop


def _split_excess_waits(nc, max_waits=1):
    from concourse import mybir

    n_split = 0
    for fn in nc.m.functions:
        for bb in fn.blocks:
            out = []
            for inst in bb.instructions:
                si = inst.sync_info
                if si is not None and si.on_wait is not None \
                        and len(si.on_wait) > max_waits:
                    waits = list(si.on_wait)
                    extra, keep = waits[:-max_waits], waits[-max_waits:]
                    for i in range(0, len(extra), max_waits):
                        out.append(
                            _make_wait_nop(inst.engine, extra[i : i + max_waits])
                        )
                    inst.sync_info = mybir.SyncInfo(
                        on_wait=keep, on_update=list(si.on_update or [])
                    )
                    n_split += 1
                out.append(inst)
            if len(out) != len(bb.instructions):
                bb.instructions[:] = out
    return n_split


# ---------------------------------------------------------------------------
# Optional NTFF profiling hook (the image's antenv stub lacks axon_hooks).
# Only used when PROBLEM_TRACE=1; grading path runs trace=False.
# ---------------------------------------------------------------------------
def _install_ntff_hook():
    so_path = "/opt/axon/libaxon_pjrt.so"
    try:
        lib = ctypes.CDLL(so_path)
    except OSError:
        return False
    if not hasattr(lib, "axon_start_nrt_profile"):
        return False
    lib.axon_start_nrt_profile.argtypes = [
        ctypes.POINTER(ctypes.c_int64),
        ctypes.c_size_t,
    ]
    lib.axon_start_nrt_profile.restype = ctypes.c_int64
    lib.axon_stop_nrt_profile.argtypes = [ctypes.c_char_p]
    lib.axon_stop_nrt_profile.restype = ctypes.c_int64

    @contextlib.contextmanager
    def _hook(output_dir, device_ids):
        import jax

        jax.devices()
        if device_ids:
            ids = (ctypes.c_int64 * len(device_ids))(*device_ids)
            rc = lib.axon_start_nrt_profile(ids, len(device_ids))
        else:
            rc = lib.axon_start_nrt_profile(None, 0)
        if rc != 0:
            raise RuntimeError(f"axon_start_nrt_profile rc={rc}")
        try:
            yield
        finally:
            n = lib.axon_stop_nrt_profile(str(output_dir).encode())
            print(f"profile: {n} file(s) written to {output_dir}", file=sys.stderr)

    mod = types.ModuleType("antenv.axon_hooks")
    mod._hook = _hook
    mod.get_axon_ntff_profile_hook = lambda: mod._hook
    mod.set_axon_ntff_profile_hook = lambda h: setattr(mod, "_hook", h)
    import antenv

    antenv.axon_hooks = mod
    sys.modules["antenv.axon_hooks"] = mod
    return True

H = 256
W = 256
HP = H + 6
WP = W + 6
C = 64
NCORES = 8
ROWS_PER_CHUNK = 2
CHUNK = ROWS_PER_CHUNK * W  # 512, one PSUM bank
NCHUNK = H // ROWS_PER_CHUNK

# tap split across engines (all 49 (dh, dw) taps):
#  - PE taps: dense merged stationaries (W1x1*s*km per tap; center tap +I
#    carries the residual), accumulated straight into the output PSUM.
#  - OFF taps run in xm-space (per-channel km scalar) on DVE (fused
#    scalar_tensor_tensor) and ACT (Copy*scale) + DVE add, into a bf16 acc;
#    one W4 matmul per chunk folds acc through the SE-scaled 1x1.
ALL_TAPS = [(dh, dw) for dh in range(7) for dw in range(7)]
DVE_TAPS = [(0, 0), (0, 2), (0, 4), (0, 6), (1, 0)]  # first is acc-init (even dw)
ACT_TAPS = [(0, 1), (0, 3), (0, 5), (1, 1), (1, 2), (1, 3), (1, 4), (1, 5),
            (1, 6), (2, 0), (2, 1), (2, 2), (2, 3), (2, 4), (2, 5), (2, 6)]
OFF_TAPS = DVE_TAPS + ACT_TAPS
PE_TAPS = [t for t in ALL_TAPS if t not in OFF_TAPS]

SUPER_ROWS = 16  # DVE/ACT superchunk rows
SUPER_PX = SUPER_ROWS * W
NSUPER = H // SUPER_ROWS
CHUNKS_PER_SUPER = SUPER_ROWS // ROWS_PER_CHUNK


def _build_nc():
    import concourse.bass as bass
    import concourse.tile as tile
    from concourse import mybir

    BF = mybir.dt.bfloat16
    FP = mybir.dt.float32

    nc = bass.Bass(target_bir_lowering=False)
    x_in = nc.dram_tensor("x", [128, HP, WP], BF, kind="ExternalInput")
    wd_in = nc.dram_tensor("wd", [128, len(PE_TAPS), 128], BF, kind="ExternalInput")
    w4_in = nc.dram_tensor("w4", [128, 128], BF, kind="ExternalInput")
    kd_in = nc.dram_tensor("kd", [128, len(OFF_TAPS)], FP, kind="ExternalInput")
    b_in = nc.dram_tensor("bias", [128, 1], FP, kind="ExternalInput")
    y_out = nc.dram_tensor("y", [128, H * W], BF, kind="ExternalOutput")

    with tile.TileContext(nc) as tc:
        with tc.tile_pool(name="xp", bufs=1) as xp, \
             tc.tile_pool(name="wp", bufs=1) as wp, \
             tc.tile_pool(name="psB", bufs=4, space="PSUM") as psB, \
             tc.tile_pool(name="accp", bufs=2) as accp, \
             tc.tile_pool(name="tmpp", bufs=2) as tmpp, \
             tc.tile_pool(name="obp", bufs=3) as obp:
            x = xp.tile([128, HP, WP], BF)
            nc.sync.dma_start(out=x, in_=x_in[:, :, :])
            wd = wp.tile([128, len(PE_TAPS), 128], BF)
            nc.sync.dma_start(out=wd, in_=wd_in[:, :, :])
            w4 = wp.tile([128, 128], BF)
            nc.sync.dma_start(out=w4, in_=w4_in[:, :])
            kd = wp.tile([128, len(OFF_TAPS)], FP)
            nc.sync.dma_start(out=kd, in_=kd_in[:, :])
            bias = wp.tile([128, 1], FP)
            nc.sync.dma_start(out=bias, in_=b_in[:, :])

            for sc in range(NSUPER):
                hs = sc * SUPER_ROWS
                acc = accp.tile([128, SUPER_ROWS, W], BF)
                for oi, (dh, dw) in enumerate(DVE_TAPS):
                    xs = x[:, hs + dh : hs + dh + SUPER_ROWS, dw : dw + W]
                    if oi == 0:
                        nc.vector.tensor_scalar_mul(
                            out=acc, in0=xs, scalar1=kd[:, 0:1]
                        )
                    else:
                        nc.vector.scalar_tensor_tensor(
                            out=acc,
                            in0=xs,
                            scalar=kd[:, oi : oi + 1],
                            in1=acc,
                            op0=mybir.AluOpType.mult,
                            op1=mybir.AluOpType.add,
                        )
                for ai, (dh, dw) in enumerate(ACT_TAPS):
                    oi = len(DVE_TAPS) + ai
                    xs = x[:, hs + dh : hs + dh + SUPER_ROWS, dw : dw + W]
                    tmp = tmpp.tile([128, SUPER_ROWS, W], BF)
                    nc.scalar.activation(
                        out=tmp,
                        in_=xs,
                        func=mybir.ActivationFunctionType.Copy,
                        scale=kd[:, oi : oi + 1],
                    )
                    nc.vector.tensor_add(out=acc, in0=acc, in1=tmp)

                for cc in range(CHUNKS_PER_SUPER):
                    h0 = hs + cc * ROWS_PER_CHUNK
                    pb = psB.tile([128, CHUNK], FP)
                    for ti, (dh, dw) in enumerate(PE_TAPS):
                        nc.tensor.matmul(
                            out=pb,
                            lhsT=wd[:, ti, :],
                            rhs=x[:, h0 + dh : h0 + dh + ROWS_PER_CHUNK, dw : dw + W],
                            start=(ti == 0),
                            stop=False,
                        )
                    nc.tensor.matmul(
                        out=pb,
                        lhsT=w4,
                        rhs=acc[:, cc * ROWS_PER_CHUNK : (cc + 1) * ROWS_PER_CHUNK, :],
                        start=False,
                        stop=True,
                    )
                    ob = obp.tile([128, CHUNK], BF)
                    nc.vector.tensor_scalar_add(out=ob, in0=pb, scalar1=bias)
                    nc.sync.dma_start(
                        out=y_out[:, h0 * W : h0 * W + CHUNK], in_=ob
                    )
    return nc


def _host_prep(x, w7, b7, w5, b5, w3, b3, fc1_w, fc1_b, fc2_w, fc2_b, w1x1, b1x1):
    import ml_dtypes

    bf16 = ml_dtypes.bfloat16

    km = w7[:, 0].astype(np.float64).copy()
    km[:, 1:6, 1:6] += w5[:, 0]
    km[:, 2:5, 2:5] += w3[:, 0]
    bm = (b7 + b5 + b3).astype(np.float64)

    n = x.shape[0]
    # SE scale: mean(xm) ~= bm + sum(km)*mean(x)  (border term ~1e-6 of out)
    mx = x.astype(np.float64).mean(axis=(2, 3))  # [N, C]
    m = bm[None, :] + km.sum(axis=(1, 2))[None, :] * mx
    y1 = np.maximum(m @ fc1_w.T.astype(np.float64) + fc1_b, 0)
    s = 1 / (1 + np.exp(-(y1 @ fc2_w.T.astype(np.float64) + fc2_b)))  # [N, C]

    w11 = w1x1[:, :, 0, 0].astype(np.float64)  # [o, c]

    # padded bf16 input per core
    xb = x.astype(bf16)
    xpad = np.zeros((NCORES, 128, HP, WP), bf16)
    xpad[:, :, 3 : 3 + H, 3 : 3 + W] = xb.reshape(NCORES, 128, H, W)

    # per-partition km scalars for the off-PE taps (same for all cores)
    kd = np.zeros((128, len(OFF_TAPS)), np.float32)
    for oi, (dh, dw) in enumerate(OFF_TAPS):
        kd[:, oi] = np.tile(km[:, dh, dw].astype(bf16).astype(np.float32), 2)

    in_maps = []
    for core in range(NCORES):
        # per-core, per-tap dense stationaries: lhsT[(n,c), (n,o)] =
        # w11[o,c]*s[n,c]*km[c,dh,dw]; center tap adds I for the residual
        s2 = s[core * 2 : core * 2 + 2]  # [2, C]
        w4 = np.zeros((128, 128), np.float64)
        bias = np.zeros((128,), np.float64)
        for ni in range(2):
            sn = s2[ni]
            w4[64 * ni : 64 * ni + 64, 64 * ni : 64 * ni + 64] = (w11 * sn[None, :]).T
            bias[64 * ni : 64 * ni + 64] = w11 @ (sn * bm) + b1x1
        wd = np.zeros((128, len(PE_TAPS), 128), np.float64)
        for ti, (dh, dw) in enumerate(PE_TAPS):
            wd[:, ti, :] = w4 * km[:, dh, dw][np.tile(np.arange(C), 2)][:, None]
        ci = PE_TAPS.index((3, 3))
        wd[:, ci, :] += np.eye(128)
        in_maps.append(
            {
                "x": xpad[core],
                "wd": wd.astype(bf16),
                "w4": w4.astype(bf16),
                "kd": kd,
                "bias": bias.astype(np.float32).reshape(128, 1),
            }
        )
    return in_maps


def kernel(x, w7, b7, w5, b5, w3, b3, fc1_w, fc1_b, fc2_w, fc2_b, w1x1, b1x1):
    args = [np.asarray(a) for a in (x, w7, b7, w5, b5, w3, b3, fc1_w, fc1_b,
                                    fc2_w, fc2_b, w1x1, b1x1)]
    in_maps = _host_prep(*args)

    if "nc" not in _CACHE:
        if os.environ.get("PROBLEM_TRACE"):
            try:
                _install_ntff_hook()
            except Exception:
                pass
        nc = _build_nc()
        _split_excess_waits(nc, max_waits=1)
        _CACHE["nc"] = nc
    nc = _CACHE["nc"]

    from concourse.bass_utils import run_bass_kernel_spmd

    trace = bool(os.environ.get("PROBLEM_TRACE"))
    res = run_bass_kernel_spmd(
        nc, in_maps, core_ids=list(range(NCORES)), trace=trace
    )
    kernel.last_result = res

    out = np.stack([r["y"] for r in res.results])  # [8, 128, H*W] bf16
    out = out.reshape(16, C, H, W).astype(np.float32)
    return out
